# revision 47
# baseline (speedup 1.0000x reference)
"""CP tensor product ('uvu' connection) kernel for Trainium2, SPMD over 8
NeuronCores.

Math per batch element b (decomposed with b = 4g + s, s in 0..3, s = 2H + s2):
  q[b,j,o]  = sum_v x2[b,j,v] w[b,o,v]          (16,64) per-b
  t1[b,r,o] = sum_d A[d,r] x1[b,d,o]            (64,64)
  t3[b,r,o] = sum_j B[j,r] q[b,j,o]             (64,64)
  out[b,c,o]= sum_r C[c,r] t1[b,r,o] t3[b,r,o]  (16,64)

Design (driven by the TimelineSim cost model, where a matmul costs
out_free_cols * 0.42ns at 1 cycle/row for bf16 and contraction size is free):
every matmul packs 128 output partitions by pairing batch elements (s2) with
static block-diagonal stationaries (a2d/bsel/c2), so PE time ~= total output
elements / 128. The per-b q contraction uses per-group block-diagonal x2
stationaries streamed pre-zero-padded from DRAM (LdWeights is free). All
inputs are bf16 (host-converted); PSUM accumulates fp32.

Hardware constraints shaping the pipeline:
  - DVE/Act ops may read at most ONE PSUM operand -> t3 is staged PSUM->SBUF
    (bf16) by the Act engine, then DVE multiplies t1(PSUM) * t3s(SBUF).
  - GPSIMD cannot access PSUM at all -> Act/DVE carry all staging; gpsimd
    only issues the output DMA queue.
  - 8 PSUM banks total: q[1] + t1[2x 1024-col tiles = 4] + t3[2] + out[1].
  - DMA: >=512B contiguous per descriptor, ~1MB per instruction to hide the
    per-instruction ~1.6us overhead; all transfers share one 360B/ns device.

Per-core DRAM layouts (g = group of 4 b's, chunk t = 8 g's, pair = 2 chunks,
super-chunk sc = 8 chunks; local batch 4096):
  x1h  [64,65536]  row 16s+d, col 64g+o : x1[4g+s,d,o]
                   (rows 0-31 serve H=0, rows 32-63 serve H=1)
  wh   [128,65536] row 32s+v,  col 64g+o : w[4g+s,o,v]
  x2bd [128,65536] row 32s+v,  col 64g+16s'+j : x2[4g+s,j,v] iff s==s', else 0
  a2d  [64,128]    blockdiag2(A) over (s2,d)x(s2,r); rows 32-63 duplicate
  bsel [128,256]   col block H: B[j,r] delta(s,2H+s2); rows 64-127 duplicate
  c2   [128,32]    blockdiag2(C^T) over (s2,r)x(s2,c)
  outh [128,32768] row 32k+16s2+c with k=2*(t%2)+H, col 512*(t//2)+64*(g%8)+o
"""
import os
os.environ.setdefault("JAX_PLATFORMS", "axon,cpu")

import numpy as np
from contextlib import ExitStack

import jax
from jax.experimental.shard_map import shard_map
from jax.sharding import Mesh, PartitionSpec, NamedSharding

import concourse.bass as bass
import concourse.bacc as bacc
import concourse.tile as tile
import concourse.mybir as mybir
from concourse._compat import with_exitstack
from concourse.bass2jax import _bass_exec_p, install_neuronx_cc_hook, partition_id_tensor

F32 = mybir.dt.float32
BF16 = mybir.dt.bfloat16
NPBF16 = mybir.dt.np(BF16)

NCORES = 8
BATCH = 32768
B_LOCAL = BATCH // NCORES   # 4096
NG = B_LOCAL // 4           # 1024 groups of 4 b's
NCHUNK = NG // 8            # 128 chunks (8 groups each)
NPAIR = NCHUNK // 2         # 64 chunk-pairs
NSC = 16                    # super-chunks (8 chunks each)
PAIRS_PER_SC = NPAIR // NSC

# Software-pipeline depths (tuned against TimelineSim):
LAG = 4        # final matmuls trail the t-stage by LAG chunks
QLEAD = 1      # q production leads consumption by QLEAD+1 pairs
SLAB_BUFS = 3  # input half-slab buffers per pool
SC_LEAD = 1    # super-chunk load lead
CONST_Q = 'scalar'   # queue for tiny stationary DMAs (keep sync queue free at start)
X2_DMA_Q = 'sync'    # queue for x2bd half-slab DMAs
TAIL_SPLIT = True    # last super-chunk: per-pair out DMAs
T3SB_BUFS = 4
Q_SPLIT = False
NWARM = 8     # dummy PE matmuls to burn through the p-state ramp at start
PIECE = 2048  # slab piece width (cols); smaller = earlier start, more DMAs
PIECE0 = 2048  # piece width for super-chunk 0 (1024 regresses: pool-slot displacement)
T3_PRIO = 0   # tile-scheduler priority boost for the t3 Act copy
MUL_PRIO = 0  # priority boost for the DVE multiply


def _emit(ctx: ExitStack, tc: tile.TileContext, outs, ins):
    from contextlib import nullcontext
    def prio(off):
        return tc.high_priority(off) if off else nullcontext()
    nc = tc.nc
    (outh,) = outs
    (x1h, wh, x2bd, a2d, bsel, c2) = ins

    const = ctx.enter_context(tc.tile_pool(name="const", bufs=1))
    a2_sb = const.tile([64, 128], BF16)
    bsel_sb = const.tile([128, 256], BF16)
    c2_sb = const.tile([128, 32], BF16)
    cq = getattr(nc, CONST_Q)
    cq.dma_start(a2_sb[:], a2d[:, :])
    cq.dma_start(bsel_sb[:], bsel[:, :])
    cq.dma_start(c2_sb[:], c2[:, :])

    x1_pool = ctx.enter_context(tc.tile_pool(name="x1", bufs=SLAB_BUFS))
    wh_pool = ctx.enter_context(tc.tile_pool(name="wh", bufs=SLAB_BUFS))
    x2_pool = ctx.enter_context(tc.tile_pool(name="x2", bufs=SLAB_BUFS))
    qsb_pool = ctx.enter_context(tc.tile_pool(name="qsb", bufs=max(2, QLEAD + 1)))
    msb_pool = ctx.enter_context(tc.tile_pool(name="msb", bufs=LAG + 1))
    t3sb_pool = ctx.enter_context(tc.tile_pool(name="t3sb", bufs=T3SB_BUFS))
    osb_pool = ctx.enter_context(tc.tile_pool(name="osb", bufs=2))
    pq = ctx.enter_context(tc.tile_pool(name="pq", bufs=1, space="PSUM"))
    pt1 = ctx.enter_context(tc.tile_pool(name="pt1", bufs=2, space="PSUM"))
    pt3 = ctx.enter_context(tc.tile_pool(name="pt3", bufs=1, space="PSUM"))
    po = ctx.enter_context(tc.tile_pool(name="po", bufs=1, space="PSUM"))

    if NWARM:
        # PE p-state warmup: harmless matmuls while the first slabs stream in
        wlhs = const.tile([1, 1], BF16)
        wrhs = const.tile([1, 512], BF16)
        nc.vector.memset(wlhs[:], 0.0)
        nc.vector.memset(wrhs[:], 0.0)
        warm_state = {"lhs": wlhs, "rhs": wrhs}
    else:
        warm_state = None

    slabs = {}   # sc -> (x1_halves, w_halves, x2_halves, q_tile)
    m_of = {}    # chunk -> m_sb tile [128,1024] (cols 0:512 H=0, 512: H=1)
    o_of = {}    # pair -> out psum tile
    osb_of = {}  # sc -> out sbuf slab

    def load_sc(sc):
        # slab pieces (pw cols each) so compute can start after one piece
        pw = PIECE0 if sc == 0 else PIECE
        x1_h, w_h, x2_h = [], [], []
        for h in range(4096 // pw):
            cw = slice(4096 * sc + pw * h, 4096 * sc + pw * (h + 1))
            x1_t = x1_pool.tile([64, pw], BF16, name="x1s")
            w_t = wh_pool.tile([128, pw], BF16, name="ws")
            x2_t = x2_pool.tile([128, pw], BF16, name="x2s")
            if sc == 0:
                # q consumes x2/w first; x1 is needed only once t1 starts
                getattr(nc, X2_DMA_Q).dma_start(x2_t[:], x2bd[:, cw])
                nc.sync.dma_start(w_t[:], wh[:, cw])
                nc.sync.dma_start(x1_t[:], x1h[:, cw])
            else:
                nc.sync.dma_start(x1_t[:], x1h[:, cw])
                nc.sync.dma_start(w_t[:], wh[:, cw])
                getattr(nc, X2_DMA_Q).dma_start(x2_t[:], x2bd[:, cw])
            x1_h.append(x1_t); w_h.append(w_t); x2_h.append(x2_t)
        q_t = qsb_pool.tile([128, 2048], BF16, name="qs")
        slabs[sc] = (x1_h, w_h, x2_h, q_t, pw)

    def emit_q(pair):
        # 16 q matmuls (one per group) for both chunks of `pair` into one
        # [128,512] psum bank (chunk parity -> partition offset 0/64), then
        # one Act copy stages it to SBUF bf16.
        (x1_h, w_h, x2_h, q_t, pw) = slabs[pair // PAIRS_PER_SC]
        ps = pq.tile([128, 512], F32, name="qps")
        if pair == 0 and warm_state:
            # PE p-state warmup into q(0)'s own bank while slabs stream in;
            # the real q matmuls below reset their regions (start=True).
            for _ in range(NWARM):
                nc.tensor.matmul(ps[0:1, :], warm_state["lhs"][:],
                                 warm_state["rhs"][:])
        for tp in range(2):            # chunk t = 2*pair + tp
            t = 2 * pair + tp
            for gg in range(8):        # group g = 8*t + gg
                gcol = 64 * ((t % 8) * 8 + gg)   # col offset within slab
                nc.tensor.matmul(
                    ps[64 * tp:64 * tp + 64, 64 * gg:64 * gg + 64],
                    x2_h[gcol // pw][:, gcol % pw:gcol % pw + 64],
                    w_h[gcol // pw][:, gcol % pw:gcol % pw + 64],
                    tile_position=(0, 64 * tp),
                )
        pb = pair % PAIRS_PER_SC
        if Q_SPLIT:
            nc.scalar.copy(q_t[:, 512 * pb:512 * pb + 256], ps[:, 0:256])
            nc.vector.tensor_scalar_mul(
                q_t[:, 512 * pb + 256:512 * (pb + 1)], ps[:, 256:512], 1.0)
        else:
            nc.scalar.copy(q_t[:, 512 * pb:512 * (pb + 1)], ps[:])

    def emit_tstage(t):
        # t1/t3 matmuls (H-interleaved), Act stages t3 to SBUF bf16, DVE
        # multiplies m = t1 * t3s.
        (x1_h, w_h, x2_h, q_t, pw) = slabs[t // 8]
        scol = 512 * (t % 8)
        x1_t = x1_h[scol // pw]
        tp = t % 2
        cw0 = scol % pw
        colw = slice(cw0, cw0 + 512)
        qcw = slice(512 * ((t % 8) // 2), 512 * ((t % 8) // 2) + 512)
        t1 = pt1.tile([128, 1024], F32, name="t1ps")
        t3 = pt3.tile([128, 1024], F32, name="t3ps")
        for H in range(2):
            nc.tensor.matmul(
                t1[:, 512 * H:512 * H + 512],
                a2_sb[32 * H:32 * H + 32, :],
                x1_t[32 * H:32 * H + 32, colw],
                tile_position=(32 * H, 0),
            )
            nc.tensor.matmul(
                t3[:, 512 * H:512 * H + 512],
                bsel_sb[64 * tp:64 * tp + 64, 128 * H:128 * (H + 1)],
                q_t[64 * tp:64 * tp + 64, qcw],
                tile_position=(64 * tp, 0),
            )
        t3s = t3sb_pool.tile([128, 1024], BF16, name="t3s")
        with prio(T3_PRIO):
            nc.scalar.copy(t3s[:], t3[:])
        m = msb_pool.tile([128, 1024], BF16, name="ms")
        with prio(MUL_PRIO):
            nc.vector.tensor_mul(m[:], t1[:], t3s[:])
        m_of[t] = m

    def emit_finals(t):
        # C contraction for chunk t into the pair's out psum bank (4 stacked
        # 32-partition strips), staged out and DMAed per super-chunk.
        pair = t // 2
        tp = t % 2
        if tp == 0:
            o_of[pair] = po.tile([128, 512], F32, name="ops")
        o_ps = o_of[pair]
        m = m_of.pop(t)
        for H in range(2):
            k = 2 * tp + H
            nc.tensor.matmul(
                o_ps[32 * k:32 * k + 32, :], c2_sb[:],
                m[:, 512 * H:512 * H + 512],
                tile_position=(0, 32 * k),
            )
        if tp == 1:
            sc = pair // PAIRS_PER_SC
            if sc not in osb_of:
                osb_of[sc] = osb_pool.tile([128, 2048], BF16, name="osb")
            pp = pair % PAIRS_PER_SC
            odst = osb_of[sc][:, 512 * pp:512 * (pp + 1)]
            ops = o_of.pop(pair)
            if pair % 2 == 0:
                nc.vector.tensor_scalar_mul(odst, ops[:], 1.0)
            else:
                nc.scalar.copy(odst, ops[:])
            if TAIL_SPLIT and sc == NSC - 1:
                nc.sync.dma_start(
                    outh[:, 2048 * sc + 512 * pp:2048 * sc + 512 * (pp + 1)],
                    osb_of[sc][:, 512 * pp:512 * (pp + 1)])
                if pp == PAIRS_PER_SC - 1:
                    osb_of.pop(sc)
            elif pp == PAIRS_PER_SC - 1:
                nc.sync.dma_start(outh[:, 2048 * sc:2048 * (sc + 1)],
                                  osb_of.pop(sc)[:])

    for k in range(SC_LEAD + 1):
        load_sc(k)
    for p in range(QLEAD + 1):
        emit_q(p)
    for t in range(NCHUNK + LAG):
        if t < NCHUNK:
            if t % 8 == 0 and t // 8 + SC_LEAD + 1 < NSC:
                load_sc(t // 8 + SC_LEAD + 1)
            if t % 2 == 0 and t // 2 + QLEAD + 1 < NPAIR:
                emit_q(t // 2 + QLEAD + 1)
            emit_tstage(t)
            if t >= LAG:
                emit_finals(t - LAG)
        else:
            emit_finals(t - LAG)


@with_exitstack
def _cp_kernel(ctx, tc, outs, ins):
    _emit(ctx, tc, outs, ins)


def build_nc():
    nc = bacc.Bacc("TRN2", target_bir_lowering=False, debug=False)
    x1h = nc.dram_tensor("x1h", [64, 65536], BF16, kind="ExternalInput").ap()
    wh = nc.dram_tensor("wh", [128, 65536], BF16, kind="ExternalInput").ap()
    x2bd = nc.dram_tensor("x2bd", [128, 65536], BF16, kind="ExternalInput").ap()
    a2d = nc.dram_tensor("a2d", [64, 128], BF16, kind="ExternalInput").ap()
    bsel = nc.dram_tensor("bsel", [128, 256], BF16, kind="ExternalInput").ap()
    c2 = nc.dram_tensor("c2", [128, 32], BF16, kind="ExternalInput").ap()
    outh = nc.dram_tensor("outh", [128, 32768], BF16, kind="ExternalOutput").ap()
    with tile.TileContext(nc, trace_sim=False) as tc:
        _cp_kernel(tc, [outh], [x1h, wh, x2bd, a2d, bsel, c2])
    nc.compile()
    return nc


def pack_inputs(x1, x2, w, A, B, C):
    """Host-side: full fp32 arrays -> per-core bf16 packed arrays (list of
    dicts keyed by dram tensor name)."""
    x1 = np.asarray(x1, np.float32)
    x2 = np.asarray(x2, np.float32)
    w = np.asarray(w, np.float32)
    A = np.asarray(A, np.float32)
    B = np.asarray(B, np.float32)
    C = np.asarray(C, np.float32)

    a2d = np.zeros((64, 128), np.float32)
    for s2 in range(2):
        a2d[16 * s2:16 * s2 + 16, 64 * s2:64 * s2 + 64] = A
    a2d[32:64] = a2d[0:32]
    bsel = np.zeros((128, 256), np.float32)
    for H in range(2):
        for s2 in range(2):
            s = 2 * H + s2
            bsel[16 * s:16 * s + 16, 128 * H + 64 * s2:128 * H + 64 * s2 + 64] = B
    bsel[64:128] = bsel[0:64]
    c2 = np.zeros((128, 32), np.float32)
    for s2 in range(2):
        c2[64 * s2:64 * s2 + 64, 16 * s2:16 * s2 + 16] = C.T

    a2d = a2d.astype(NPBF16)
    bsel = bsel.astype(NPBF16)
    c2 = c2.astype(NPBF16)

    in_maps = []
    for cidx in range(NCORES):
        sl = slice(cidx * B_LOCAL, (cidx + 1) * B_LOCAL)
        x1c = x1[sl].reshape(NG, 4, 16, 64)          # [g,s,d,o]
        x1h = np.ascontiguousarray(
            x1c.transpose(1, 2, 0, 3)).reshape(64, 65536).astype(NPBF16)
        wc = w[sl].reshape(NG, 4, 64, 32)            # [g,s,o,v]
        wh = np.ascontiguousarray(
            wc.transpose(1, 3, 0, 2)).reshape(128, 65536).astype(NPBF16)
        x2c = x2[sl].reshape(NG, 4, 16, 32)          # [g,s,j,v]
        x2t = x2c.transpose(1, 3, 0, 2)              # [s,v,g,j]
        x2bd = np.zeros((128, 65536), NPBF16)
        x2v = x2bd.reshape(4, 32, NG, 64)            # [s,v,g,(s',j)]
        for s in range(4):
            x2v[s, :, :, 16 * s:16 * s + 16] = x2t[s].astype(NPBF16)
        in_maps.append({"x1h": x1h, "wh": wh, "x2bd": x2bd,
                        "a2d": a2d, "bsel": bsel, "c2": c2})
    return in_maps


def unpack_out(outh_all):
    """outh_all: (NCORES*128, 32768) bf16 -> (BATCH, 16, 64) fp32."""
    out = np.empty((BATCH, 16, 64), np.float32)
    for cidx in range(NCORES):
        oc = np.asarray(outh_all[cidx * 128:(cidx + 1) * 128]).astype(np.float32)
        # rows: [tpar(2), H(2), s2(2), c(16)]; cols: [pb(64), gsub(8), o(64)]
        v = oc.reshape(2, 2, 2, 16, 64, 8, 64)
        # b = ((pb*2 + tpar)*8 + gsub)*4 + 2H + s2
        v = v.transpose(4, 0, 5, 1, 2, 3, 6)  # [pb,tpar,gsub,H,s2,c,o]
        out[cidx * B_LOCAL:(cidx + 1) * B_LOCAL] = v.reshape(B_LOCAL, 16, 64)
    return out


class _SpmdRunner:
    """Persistent jitted SPMD executor over the 8 NeuronCores."""

    def __init__(self, nc, n_cores=NCORES):
        install_neuronx_cc_hook()
        self.nc = nc
        self.n_cores = n_cores
        pid_name = nc.partition_id_tensor.name if nc.partition_id_tensor else None

        in_names, out_names, out_avals, zero_outs = [], [], [], []
        for alloc in nc.m.functions[0].allocations:
            if not isinstance(alloc, mybir.MemoryLocationSet):
                continue
            name = alloc.memorylocations[0].name
            if alloc.kind == "ExternalInput":
                if name != pid_name:
                    in_names.append(name)
            elif alloc.kind == "ExternalOutput":
                out_names.append(name)
                shape = tuple(alloc.tensor_shape)
                dtype = mybir.dt.np(alloc.dtype)
                out_avals.append(jax.core.ShapedArray(shape, dtype))
                zero_outs.append(np.zeros(shape, dtype))
        self.in_names, self.out_names = in_names, out_names
        self.out_avals, self.zero_outs = out_avals, zero_outs
        n_params = len(in_names)
        all_names = tuple(in_names + out_names + ([pid_name] if pid_name else []))

        def _body(*args):
            operands = list(args)
            if pid_name is not None:
                operands.append(partition_id_tensor())
            outs = _bass_exec_p.bind(
                *operands,
                out_avals=tuple(out_avals),
                in_names=all_names,
                out_names=tuple(out_names),
                lowering_input_output_aliases=(),
                sim_require_finite=True,
                sim_require_nnan=True,
                nc=nc,
            )
            return tuple(outs)

        devices = jax.devices()[:n_cores]
        self.mesh = Mesh(np.asarray(devices), ("core",))
        self.sharding = NamedSharding(self.mesh, PartitionSpec("core"))
        n_out = len(out_names)
        donate = tuple(range(n_params, n_params + n_out))
        self.jitted = jax.jit(
            shard_map(_body, mesh=self.mesh,
                      in_specs=(PartitionSpec("core"),) * (n_params + n_out),
                      out_specs=(PartitionSpec("core"),) * n_out,
                      check_rep=False),
            donate_argnums=donate, keep_unused=True,
        )

    def stage_inputs(self, in_maps):
        per_core = [[np.asarray(m[name]) for name in self.in_names] for m in in_maps]
        concat = [np.concatenate([per_core[c][i] for c in range(self.n_cores)], axis=0)
                  for i in range(len(self.in_names))]
        return [jax.device_put(a, self.sharding) for a in concat]

    def stage_zeros(self):
        zs = [np.zeros((self.n_cores * z.shape[0], *z.shape[1:]), z.dtype)
              for z in self.zero_outs]
        return [jax.device_put(z, self.sharding) for z in zs]

    def run(self, dev_inputs, dev_zeros=None):
        if dev_zeros is None:
            dev_zeros = self.stage_zeros()
        outs = self.jitted(*dev_inputs, *dev_zeros)
        jax.block_until_ready(outs)
        return outs

    def unshard_out(self, outs):
        i = self.out_names.index("outh")
        return unpack_out(np.asarray(outs[i]))


_RUNNER = None


def _get_runner():
    global _RUNNER
    if _RUNNER is None:
        nc = build_nc()
        _RUNNER = _SpmdRunner(nc, NCORES)
    return _RUNNER


def kernel(x1, x2, w, A, B, C):
    """Full-input entry point. Shards batch across 8 NeuronCores, runs the
    Bass kernel, gathers the full output (32768, 16, 64) float32."""
    runner = _get_runner()
    in_maps = pack_inputs(x1, x2, w, A, B, C)
    dev_in = runner.stage_inputs(in_maps)
    outs = runner.run(dev_in)
    return runner.unshard_out(outs)


# revision 49
# speedup vs baseline: 1.0012x; 1.0012x over previous
"""CP tensor product ('uvu' connection) kernel for Trainium2, SPMD over 8
NeuronCores.

Math per batch element b (decomposed with b = 4g + s, s in 0..3, s = 2H + s2):
  q[b,j,o]  = sum_v x2[b,j,v] w[b,o,v]          (16,64) per-b
  t1[b,r,o] = sum_d A[d,r] x1[b,d,o]            (64,64)
  t3[b,r,o] = sum_j B[j,r] q[b,j,o]             (64,64)
  out[b,c,o]= sum_r C[c,r] t1[b,r,o] t3[b,r,o]  (16,64)

Design (driven by the TimelineSim cost model, where a matmul costs
out_free_cols * 0.42ns at 1 cycle/row for bf16 and contraction size is free):
every matmul packs 128 output partitions by pairing batch elements (s2) with
static block-diagonal stationaries (a2d/bsel/c2), so PE time ~= total output
elements / 128. The per-b q contraction uses per-group block-diagonal x2
stationaries streamed pre-zero-padded from DRAM (LdWeights is free). All
inputs are bf16 (host-converted); PSUM accumulates fp32.

Hardware constraints shaping the pipeline:
  - DVE/Act ops may read at most ONE PSUM operand -> t3 is staged PSUM->SBUF
    (bf16) by the Act engine, then DVE multiplies t1(PSUM) * t3s(SBUF).
  - GPSIMD cannot access PSUM at all -> Act/DVE carry all staging; gpsimd
    only issues the output DMA queue.
  - 8 PSUM banks total: q[1] + t1[2x 1024-col tiles = 4] + t3[2] + out[1].
  - DMA: >=512B contiguous per descriptor, ~1MB per instruction to hide the
    per-instruction ~1.6us overhead; all transfers share one 360B/ns device.

Per-core DRAM layouts (g = group of 4 b's, chunk t = 8 g's, pair = 2 chunks,
super-chunk sc = 8 chunks; local batch 4096):
  x1h  [64,65536]  row 16s+d, col 64g+o : x1[4g+s,d,o]
                   (rows 0-31 serve H=0, rows 32-63 serve H=1)
  wh   [128,65536] row 32s+v,  col 64g+o : w[4g+s,o,v]
  x2bd [128,65536] row 32s+v,  col 64g+16s'+j : x2[4g+s,j,v] iff s==s', else 0
  a2d  [64,128]    blockdiag2(A) over (s2,d)x(s2,r); rows 32-63 duplicate
  bsel [128,256]   col block H: B[j,r] delta(s,2H+s2); rows 64-127 duplicate
  c2   [128,32]    blockdiag2(C^T) over (s2,r)x(s2,c)
  outh [128,32768] row 32k+16s2+c with k=2*(t%2)+H, col 512*(t//2)+64*(g%8)+o
"""
import os
os.environ.setdefault("JAX_PLATFORMS", "axon,cpu")

import numpy as np
from contextlib import ExitStack

import jax
from jax.experimental.shard_map import shard_map
from jax.sharding import Mesh, PartitionSpec, NamedSharding

import concourse.bass as bass
import concourse.bacc as bacc
import concourse.tile as tile
import concourse.mybir as mybir
from concourse._compat import with_exitstack
from concourse.bass2jax import _bass_exec_p, install_neuronx_cc_hook, partition_id_tensor

F32 = mybir.dt.float32
BF16 = mybir.dt.bfloat16
NPBF16 = mybir.dt.np(BF16)

NCORES = 8
BATCH = 32768
B_LOCAL = BATCH // NCORES   # 4096
NG = B_LOCAL // 4           # 1024 groups of 4 b's
NCHUNK = NG // 8            # 128 chunks (8 groups each)
NPAIR = NCHUNK // 2         # 64 chunk-pairs
NSC = 16                    # super-chunks (8 chunks each)
PAIRS_PER_SC = NPAIR // NSC

# Software-pipeline depths (tuned against TimelineSim):
LAG = 4        # final matmuls trail the t-stage by LAG chunks
QLEAD = 0      # q production leads consumption by QLEAD+1 pairs
SLAB_BUFS = 3  # input half-slab buffers per pool
SC_LEAD = 1    # super-chunk load lead
CONST_Q = 'scalar'   # queue for tiny stationary DMAs (keep sync queue free at start)
X2_DMA_Q = 'sync'    # queue for x2bd half-slab DMAs
TAIL_SPLIT = True    # last super-chunk: per-pair out DMAs
T3SB_BUFS = 4
Q_SPLIT = False
NWARM = 8     # dummy PE matmuls to burn through the p-state ramp at start
PIECE = 2048  # slab piece width (cols); smaller = earlier start, more DMAs
PIECE0 = 2048  # piece width for super-chunk 0 (1024 regresses: pool-slot displacement)
T3_PRIO = 0   # tile-scheduler priority boost for the t3 Act copy
MUL_PRIO = 0  # priority boost for the DVE multiply
OSB_BUFS = 2


def _emit(ctx: ExitStack, tc: tile.TileContext, outs, ins):
    from contextlib import nullcontext
    def prio(off):
        return tc.high_priority(off) if off else nullcontext()
    nc = tc.nc
    (outh,) = outs
    (x1h, wh, x2bd, a2d, bsel, c2) = ins

    const = ctx.enter_context(tc.tile_pool(name="const", bufs=1))
    a2_sb = const.tile([64, 128], BF16)
    bsel_sb = const.tile([128, 256], BF16)
    c2_sb = const.tile([128, 32], BF16)
    cq = getattr(nc, CONST_Q)
    cq.dma_start(a2_sb[:], a2d[:, :])
    cq.dma_start(bsel_sb[:], bsel[:, :])
    cq.dma_start(c2_sb[:], c2[:, :])

    x1_pool = ctx.enter_context(tc.tile_pool(name="x1", bufs=SLAB_BUFS))
    wh_pool = ctx.enter_context(tc.tile_pool(name="wh", bufs=SLAB_BUFS))
    x2_pool = ctx.enter_context(tc.tile_pool(name="x2", bufs=SLAB_BUFS))
    qsb_pool = ctx.enter_context(tc.tile_pool(name="qsb", bufs=max(2, QLEAD + 1)))
    msb_pool = ctx.enter_context(tc.tile_pool(name="msb", bufs=LAG + 1))
    t3sb_pool = ctx.enter_context(tc.tile_pool(name="t3sb", bufs=T3SB_BUFS))
    osb_pool = ctx.enter_context(tc.tile_pool(name="osb", bufs=OSB_BUFS))
    pq = ctx.enter_context(tc.tile_pool(name="pq", bufs=1, space="PSUM"))
    pt1 = ctx.enter_context(tc.tile_pool(name="pt1", bufs=2, space="PSUM"))
    pt3 = ctx.enter_context(tc.tile_pool(name="pt3", bufs=1, space="PSUM"))
    po = ctx.enter_context(tc.tile_pool(name="po", bufs=1, space="PSUM"))

    if NWARM:
        # PE p-state warmup: harmless matmuls while the first slabs stream in
        wlhs = const.tile([1, 1], BF16)
        wrhs = const.tile([1, 512], BF16)
        nc.vector.memset(wlhs[:], 0.0)
        nc.vector.memset(wrhs[:], 0.0)
        warm_state = {"lhs": wlhs, "rhs": wrhs}
    else:
        warm_state = None

    slabs = {}   # sc -> (x1_halves, w_halves, x2_halves, q_tile)
    m_of = {}    # chunk -> m_sb tile [128,1024] (cols 0:512 H=0, 512: H=1)
    o_of = {}    # pair -> out psum tile
    osb_of = {}  # sc -> out sbuf slab

    def load_sc(sc):
        # slab pieces (pw cols each) so compute can start after one piece
        pw = PIECE0 if sc == 0 else PIECE
        x1_h, w_h, x2_h = [], [], []
        for h in range(4096 // pw):
            cw = slice(4096 * sc + pw * h, 4096 * sc + pw * (h + 1))
            x1_t = x1_pool.tile([64, pw], BF16, name="x1s")
            w_t = wh_pool.tile([128, pw], BF16, name="ws")
            x2_t = x2_pool.tile([128, pw], BF16, name="x2s")
            if sc == 0:
                # q consumes x2/w first; x1 is needed only once t1 starts
                getattr(nc, X2_DMA_Q).dma_start(x2_t[:], x2bd[:, cw])
                nc.sync.dma_start(w_t[:], wh[:, cw])
                nc.sync.dma_start(x1_t[:], x1h[:, cw])
            else:
                nc.sync.dma_start(x1_t[:], x1h[:, cw])
                nc.sync.dma_start(w_t[:], wh[:, cw])
                getattr(nc, X2_DMA_Q).dma_start(x2_t[:], x2bd[:, cw])
            x1_h.append(x1_t); w_h.append(w_t); x2_h.append(x2_t)
        q_t = qsb_pool.tile([128, 2048], BF16, name="qs")
        slabs[sc] = (x1_h, w_h, x2_h, q_t, pw)

    def emit_q(pair):
        # 16 q matmuls (one per group) for both chunks of `pair` into one
        # [128,512] psum bank (chunk parity -> partition offset 0/64), then
        # one Act copy stages it to SBUF bf16.
        (x1_h, w_h, x2_h, q_t, pw) = slabs[pair // PAIRS_PER_SC]
        ps = pq.tile([128, 512], F32, name="qps")
        if pair == 0 and warm_state:
            # PE p-state warmup into q(0)'s own bank while slabs stream in;
            # the real q matmuls below reset their regions (start=True).
            for _ in range(NWARM):
                nc.tensor.matmul(ps[0:1, :], warm_state["lhs"][:],
                                 warm_state["rhs"][:])
        for tp in range(2):            # chunk t = 2*pair + tp
            t = 2 * pair + tp
            for gg in range(8):        # group g = 8*t + gg
                gcol = 64 * ((t % 8) * 8 + gg)   # col offset within slab
                nc.tensor.matmul(
                    ps[64 * tp:64 * tp + 64, 64 * gg:64 * gg + 64],
                    x2_h[gcol // pw][:, gcol % pw:gcol % pw + 64],
                    w_h[gcol // pw][:, gcol % pw:gcol % pw + 64],
                    tile_position=(0, 64 * tp),
                )
        pb = pair % PAIRS_PER_SC
        if Q_SPLIT:
            nc.scalar.copy(q_t[:, 512 * pb:512 * pb + 256], ps[:, 0:256])
            nc.vector.tensor_scalar_mul(
                q_t[:, 512 * pb + 256:512 * (pb + 1)], ps[:, 256:512], 1.0)
        else:
            nc.scalar.copy(q_t[:, 512 * pb:512 * (pb + 1)], ps[:])

    def emit_tstage(t):
        # t1/t3 matmuls (H-interleaved), Act stages t3 to SBUF bf16, DVE
        # multiplies m = t1 * t3s.
        (x1_h, w_h, x2_h, q_t, pw) = slabs[t // 8]
        scol = 512 * (t % 8)
        x1_t = x1_h[scol // pw]
        tp = t % 2
        cw0 = scol % pw
        colw = slice(cw0, cw0 + 512)
        qcw = slice(512 * ((t % 8) // 2), 512 * ((t % 8) // 2) + 512)
        t1 = pt1.tile([128, 1024], F32, name="t1ps")
        t3 = pt3.tile([128, 1024], F32, name="t3ps")
        for H in range(2):
            nc.tensor.matmul(
                t1[:, 512 * H:512 * H + 512],
                a2_sb[32 * H:32 * H + 32, :],
                x1_t[32 * H:32 * H + 32, colw],
                tile_position=(32 * H, 0),
            )
            nc.tensor.matmul(
                t3[:, 512 * H:512 * H + 512],
                bsel_sb[64 * tp:64 * tp + 64, 128 * H:128 * (H + 1)],
                q_t[64 * tp:64 * tp + 64, qcw],
                tile_position=(64 * tp, 0),
            )
        t3s = t3sb_pool.tile([128, 1024], BF16, name="t3s")
        with prio(T3_PRIO):
            nc.scalar.copy(t3s[:], t3[:])
        m = msb_pool.tile([128, 1024], BF16, name="ms")
        with prio(MUL_PRIO):
            nc.vector.tensor_mul(m[:], t1[:], t3s[:])
        m_of[t] = m

    def emit_finals(t):
        # C contraction for chunk t into the pair's out psum bank (4 stacked
        # 32-partition strips), staged out and DMAed per super-chunk.
        pair = t // 2
        tp = t % 2
        if tp == 0:
            o_of[pair] = po.tile([128, 512], F32, name="ops")
        o_ps = o_of[pair]
        m = m_of.pop(t)
        for H in range(2):
            k = 2 * tp + H
            nc.tensor.matmul(
                o_ps[32 * k:32 * k + 32, :], c2_sb[:],
                m[:, 512 * H:512 * H + 512],
                tile_position=(0, 32 * k),
            )
        if tp == 1:
            sc = pair // PAIRS_PER_SC
            if sc not in osb_of:
                osb_of[sc] = osb_pool.tile([128, 2048], BF16, name="osb")
            pp = pair % PAIRS_PER_SC
            odst = osb_of[sc][:, 512 * pp:512 * (pp + 1)]
            ops = o_of.pop(pair)
            if pair % 2 == 0:
                nc.vector.tensor_scalar_mul(odst, ops[:], 1.0)
            else:
                nc.scalar.copy(odst, ops[:])
            if TAIL_SPLIT and sc == NSC - 1:
                nc.sync.dma_start(
                    outh[:, 2048 * sc + 512 * pp:2048 * sc + 512 * (pp + 1)],
                    osb_of[sc][:, 512 * pp:512 * (pp + 1)])
                if pp == PAIRS_PER_SC - 1:
                    osb_of.pop(sc)
            elif pp == PAIRS_PER_SC - 1:
                nc.sync.dma_start(outh[:, 2048 * sc:2048 * (sc + 1)],
                                  osb_of.pop(sc)[:])

    for k in range(SC_LEAD + 1):
        load_sc(k)
    for p in range(QLEAD + 1):
        emit_q(p)
    for t in range(NCHUNK + LAG):
        if t < NCHUNK:
            if t % 8 == 0 and t // 8 + SC_LEAD + 1 < NSC:
                load_sc(t // 8 + SC_LEAD + 1)
            if t % 2 == 0 and t // 2 + QLEAD + 1 < NPAIR:
                emit_q(t // 2 + QLEAD + 1)
            emit_tstage(t)
            if t >= LAG:
                emit_finals(t - LAG)
        else:
            emit_finals(t - LAG)


@with_exitstack
def _cp_kernel(ctx, tc, outs, ins):
    _emit(ctx, tc, outs, ins)


def build_nc():
    nc = bacc.Bacc("TRN2", target_bir_lowering=False, debug=False)
    x1h = nc.dram_tensor("x1h", [64, 65536], BF16, kind="ExternalInput").ap()
    wh = nc.dram_tensor("wh", [128, 65536], BF16, kind="ExternalInput").ap()
    x2bd = nc.dram_tensor("x2bd", [128, 65536], BF16, kind="ExternalInput").ap()
    a2d = nc.dram_tensor("a2d", [64, 128], BF16, kind="ExternalInput").ap()
    bsel = nc.dram_tensor("bsel", [128, 256], BF16, kind="ExternalInput").ap()
    c2 = nc.dram_tensor("c2", [128, 32], BF16, kind="ExternalInput").ap()
    outh = nc.dram_tensor("outh", [128, 32768], BF16, kind="ExternalOutput").ap()
    with tile.TileContext(nc, trace_sim=False) as tc:
        _cp_kernel(tc, [outh], [x1h, wh, x2bd, a2d, bsel, c2])
    nc.compile()
    return nc


def pack_inputs(x1, x2, w, A, B, C):
    """Host-side: full fp32 arrays -> per-core bf16 packed arrays (list of
    dicts keyed by dram tensor name)."""
    x1 = np.asarray(x1, np.float32)
    x2 = np.asarray(x2, np.float32)
    w = np.asarray(w, np.float32)
    A = np.asarray(A, np.float32)
    B = np.asarray(B, np.float32)
    C = np.asarray(C, np.float32)

    a2d = np.zeros((64, 128), np.float32)
    for s2 in range(2):
        a2d[16 * s2:16 * s2 + 16, 64 * s2:64 * s2 + 64] = A
    a2d[32:64] = a2d[0:32]
    bsel = np.zeros((128, 256), np.float32)
    for H in range(2):
        for s2 in range(2):
            s = 2 * H + s2
            bsel[16 * s:16 * s + 16, 128 * H + 64 * s2:128 * H + 64 * s2 + 64] = B
    bsel[64:128] = bsel[0:64]
    c2 = np.zeros((128, 32), np.float32)
    for s2 in range(2):
        c2[64 * s2:64 * s2 + 64, 16 * s2:16 * s2 + 16] = C.T

    a2d = a2d.astype(NPBF16)
    bsel = bsel.astype(NPBF16)
    c2 = c2.astype(NPBF16)

    in_maps = []
    for cidx in range(NCORES):
        sl = slice(cidx * B_LOCAL, (cidx + 1) * B_LOCAL)
        x1c = x1[sl].reshape(NG, 4, 16, 64)          # [g,s,d,o]
        x1h = np.ascontiguousarray(
            x1c.transpose(1, 2, 0, 3)).reshape(64, 65536).astype(NPBF16)
        wc = w[sl].reshape(NG, 4, 64, 32)            # [g,s,o,v]
        wh = np.ascontiguousarray(
            wc.transpose(1, 3, 0, 2)).reshape(128, 65536).astype(NPBF16)
        x2c = x2[sl].reshape(NG, 4, 16, 32)          # [g,s,j,v]
        x2t = x2c.transpose(1, 3, 0, 2)              # [s,v,g,j]
        x2bd = np.zeros((128, 65536), NPBF16)
        x2v = x2bd.reshape(4, 32, NG, 64)            # [s,v,g,(s',j)]
        for s in range(4):
            x2v[s, :, :, 16 * s:16 * s + 16] = x2t[s].astype(NPBF16)
        in_maps.append({"x1h": x1h, "wh": wh, "x2bd": x2bd,
                        "a2d": a2d, "bsel": bsel, "c2": c2})
    return in_maps


def unpack_out(outh_all):
    """outh_all: (NCORES*128, 32768) bf16 -> (BATCH, 16, 64) fp32."""
    out = np.empty((BATCH, 16, 64), np.float32)
    for cidx in range(NCORES):
        oc = np.asarray(outh_all[cidx * 128:(cidx + 1) * 128]).astype(np.float32)
        # rows: [tpar(2), H(2), s2(2), c(16)]; cols: [pb(64), gsub(8), o(64)]
        v = oc.reshape(2, 2, 2, 16, 64, 8, 64)
        # b = ((pb*2 + tpar)*8 + gsub)*4 + 2H + s2
        v = v.transpose(4, 0, 5, 1, 2, 3, 6)  # [pb,tpar,gsub,H,s2,c,o]
        out[cidx * B_LOCAL:(cidx + 1) * B_LOCAL] = v.reshape(B_LOCAL, 16, 64)
    return out


class _SpmdRunner:
    """Persistent jitted SPMD executor over the 8 NeuronCores."""

    def __init__(self, nc, n_cores=NCORES):
        install_neuronx_cc_hook()
        self.nc = nc
        self.n_cores = n_cores
        pid_name = nc.partition_id_tensor.name if nc.partition_id_tensor else None

        in_names, out_names, out_avals, zero_outs = [], [], [], []
        for alloc in nc.m.functions[0].allocations:
            if not isinstance(alloc, mybir.MemoryLocationSet):
                continue
            name = alloc.memorylocations[0].name
            if alloc.kind == "ExternalInput":
                if name != pid_name:
                    in_names.append(name)
            elif alloc.kind == "ExternalOutput":
                out_names.append(name)
                shape = tuple(alloc.tensor_shape)
                dtype = mybir.dt.np(alloc.dtype)
                out_avals.append(jax.core.ShapedArray(shape, dtype))
                zero_outs.append(np.zeros(shape, dtype))
        self.in_names, self.out_names = in_names, out_names
        self.out_avals, self.zero_outs = out_avals, zero_outs
        n_params = len(in_names)
        all_names = tuple(in_names + out_names + ([pid_name] if pid_name else []))

        def _body(*args):
            operands = list(args)
            if pid_name is not None:
                operands.append(partition_id_tensor())
            outs = _bass_exec_p.bind(
                *operands,
                out_avals=tuple(out_avals),
                in_names=all_names,
                out_names=tuple(out_names),
                lowering_input_output_aliases=(),
                sim_require_finite=True,
                sim_require_nnan=True,
                nc=nc,
            )
            return tuple(outs)

        devices = jax.devices()[:n_cores]
        self.mesh = Mesh(np.asarray(devices), ("core",))
        self.sharding = NamedSharding(self.mesh, PartitionSpec("core"))
        n_out = len(out_names)
        donate = tuple(range(n_params, n_params + n_out))
        self.jitted = jax.jit(
            shard_map(_body, mesh=self.mesh,
                      in_specs=(PartitionSpec("core"),) * (n_params + n_out),
                      out_specs=(PartitionSpec("core"),) * n_out,
                      check_rep=False),
            donate_argnums=donate, keep_unused=True,
        )

    def stage_inputs(self, in_maps):
        per_core = [[np.asarray(m[name]) for name in self.in_names] for m in in_maps]
        concat = [np.concatenate([per_core[c][i] for c in range(self.n_cores)], axis=0)
                  for i in range(len(self.in_names))]
        return [jax.device_put(a, self.sharding) for a in concat]

    def stage_zeros(self):
        zs = [np.zeros((self.n_cores * z.shape[0], *z.shape[1:]), z.dtype)
              for z in self.zero_outs]
        return [jax.device_put(z, self.sharding) for z in zs]

    def run(self, dev_inputs, dev_zeros=None):
        if dev_zeros is None:
            dev_zeros = self.stage_zeros()
        outs = self.jitted(*dev_inputs, *dev_zeros)
        jax.block_until_ready(outs)
        return outs

    def unshard_out(self, outs):
        i = self.out_names.index("outh")
        return unpack_out(np.asarray(outs[i]))


_RUNNER = None


def _get_runner():
    global _RUNNER
    if _RUNNER is None:
        nc = build_nc()
        _RUNNER = _SpmdRunner(nc, NCORES)
    return _RUNNER


def kernel(x1, x2, w, A, B, C):
    """Full-input entry point. Shards batch across 8 NeuronCores, runs the
    Bass kernel, gathers the full output (32768, 16, 64) float32."""
    runner = _get_runner()
    in_maps = pack_inputs(x1, x2, w, A, B, C)
    dev_in = runner.stage_inputs(in_maps)
    outs = runner.run(dev_in)
    return runner.unshard_out(outs)
